# revision 10
# baseline (speedup 1.0000x reference)
"""Block-sparse linear y = x @ W^T + bias on 8 Trainium2 NeuronCores.

W [4096, 4096] has K=1024 dense 64x64 tiles at (row_idx[k], col_idx[k]) on a
64x64 block grid. Data-parallel over tokens: each core gets 512 rows of x and
all blocks, computing yT_local [4096, 512] = W @ x_local^T + bias.

Per-core device layout:
  - xT in SBUF (bf16) as one [128, NSLOT*512] image: input column blocks are
    paired into "slots"; the first member of slot s lives at partitions 0-63,
    the second at 64-127, free offset s*512. Slots are ordered by first use
    across the chunk schedule, and the image is pre-arranged on the host so
    each DMA moves [128, 2048] slabs with 4 KB contiguous runs per partition.
  - all block weights SBUF-resident as one [128, wtot*64] bf16 image of
    B_k^T tiles (parity-0 member of each chunk on partitions 0-63, parity-1
    on 64-127), loaded once in 8 chunk-aligned ~1 MB DMAs.
  - output block-rows are processed in 32 PSUM generations ("chunks") of 2
    rows (one per 64-partition half). parity-0 contributions accumulate in
    psum bank 2k, parity-1 in bank 2k+1 (a PSUM accumulation group must keep
    one tile_position); the four (parity, half) quadrants of the 128x128 PE
    run concurrently via tile_position. Banks rotate 4-deep.
  - drain: ACT adds bias while copying the even bank to SBUF, DVE adds the
    odd bank and writes bf16; every 4 chunks one [128, 2048] bf16 DMA (on
    the scalar HWDGE queue, separate from the inbound sync queue) stores to
    yT in DRAM. The host converts back to f32.

All inbound DMAs go through the sync HWDGE queue, which drains FIFO: x and
weight batches are interleaved so compute prerequisites land just in time
and the PE, once started, never starves (the PE p-state ramp 0.65->1.2->2.4
GHz resets on idle, so continuous busy = full clock).

The schedule is specialized on the host from row_idx/col_idx (duplicate
(r,c) blocks are pre-summed); all matmul FLOPs run on the PE.

This toolchain's walrus rejects >1 semaphore wait per instruction, so after
Tile scheduling we split excess waits onto same-engine NoOps.
"""

import numpy as np
import ml_dtypes

N_TOK, IN_F, OUT_F, BS, NCORES = 4096, 4096, 4096, 64, 8
NL = N_TOK // NCORES            # tokens per core (512)
GRID = OUT_F // BS              # 64 block-rows / block-cols
NCHUNK = 32                     # psum generations, 2 block-rows each
XB = 4                          # slots per x DMA batch
WB = 4                          # chunks per weight DMA batch
OB = 4                          # chunks per output DMA batch
NWARM = 40                      # zero-tile warmup matmuls (PE ramp)

_CACHE = {}


def _plan(row_idx, col_idx):
    from collections import OrderedDict

    K = int(row_idx.shape[0])
    cells = OrderedDict()
    for k in range(K):
        cells.setdefault((int(row_idx[k]), int(col_idx[k])), []).append(k)
    rows = {r: [] for r in range(GRID)}
    for (r, c) in cells:
        rows[r].append(c)
    counts = {r: len(rows[r]) for r in range(GRID)}

    # chunk 0: the row pair with the smallest column-set union, so the first
    # chunk's x and weight prerequisites are minimal
    best = None
    for r1 in range(GRID):
        for r2 in range(r1 + 1, GRID):
            u = len(set(rows[r1]) | set(rows[r2]))
            if best is None or u < best[0]:
                best = (u, r1, r2)
    first_pair = [best[1], best[2]]

    # remaining 31 chunks x 2 rows, balanced by block count
    rest = [r for r in range(GRID) if r not in first_pair]
    order = sorted(rest, key=lambda r: -counts[r])
    per_chunk = GRID // NCHUNK
    chunk_rows = [[] for _ in range(NCHUNK - 1)]
    load = [0] * (NCHUNK - 1)
    for r in order:
        cands = [i for i in range(NCHUNK - 1) if len(chunk_rows[i]) < per_chunk]
        i = min(cands, key=lambda j: (load[j], len(chunk_rows[j])))
        chunk_rows[i].append(r)
        load[i] += counts[r]

    # order chunks greedily by fewest new columns, compressing the early
    # x-prefetch demand so compute can start while most of x is in flight
    cur = set(c for r in first_pair for c in rows[r])
    rem = list(chunk_rows)
    ordered = [first_pair]
    while rem:
        nxt = min(rem, key=lambda rs: len({c for r in rs for c in rows[r]}
                                          - cur))
        rem.remove(nxt)
        ordered.append(nxt)
        cur |= {c for r in nxt for c in rows[r]}
    chunk_rows = ordered

    # slot assignment: pair columns in order of first use across the chunk
    # schedule, so early chunks only need the head of the xT image
    seen, pending, pair = set(), [], []
    for rs in chunk_rows:
        for c in sorted({c for r in rs for c in rows[r]}):
            if c not in seen:
                seen.add(c)
                pending.append(c)
            if len(pending) >= 2:
                pair.append((pending.pop(0), pending.pop(0)))
    if pending:
        c = pending.pop()
        pair.append((c, c))
    slot, par = {}, {}
    for s, (a, b) in enumerate(pair):
        slot[a], par[a] = s, 0
        if b != a:
            slot[b], par[b] = s, 1
    nslot = len(pair)

    chunks = []
    for rs in chunk_rows:
        rs = sorted(rs, key=lambda r: -counts[r])
        # one row to the top half, one to the bottom, balancing parity loads
        top, bot = [], []
        Et = Ot = Eb = Ob = 0
        for r in rs:
            e = sum(1 for c in rows[r] if par[c] == 0)
            o = counts[r] - e
            ct = abs(Et + e - Eb) + abs(Ot + o - Ob)
            cb = abs(Et - Eb - e) + abs(Ot - Ob - o)
            if len(top) < 1 and (len(bot) >= 1 or ct <= cb):
                top.append(r); Et += e; Ot += o
            else:
                bot.append(r); Eb += e; Ob += o
        regions = {}            # (pair, half) -> r
        for k, r in enumerate(top):
            regions[(k, 0)] = r
        for k, r in enumerate(bot):
            regions[(k, 1)] = r
        a, b = regions.get((0, 0)), regions.get((0, 1))
        if a is not None and b is not None and a > b:
            regions[(0, 0)], regions[(0, 1)] = b, a
        rloc = {r: kh for kh, r in regions.items()}

        # emission queues by quadrant (parity, half), entries sorted by slot
        queues = {(p, h): [] for p in (0, 1) for h in (0, 1)}
        for r in rs:
            k, h = rloc[r]
            for c in rows[r]:
                queues[(par[c], h)].append((r, c))
        for q in queues.values():
            q.sort(key=lambda rc: slot[rc[1]])

        cycle = [(0, 0), (1, 1), (1, 0), (0, 1)]
        ent = []
        qi = {kq: 0 for kq in queues}
        total = sum(len(q) for q in queues.values())
        while len(ent) < total:
            prog = False
            for kq in cycle:
                q = queues[kq]
                if qi[kq] < len(q):
                    ent.append(q[qi[kq]])
                    qi[kq] += 1
                    prog = True
            assert prog
        # start/stop per accumulator = (region, parity)
        first, last = {}, {}
        for i, (r, c) in enumerate(ent):
            acc = (r, par[c])
            first.setdefault(acc, i)
            last[acc] = i
        entries = []
        for i, (r, c) in enumerate(ent):
            k, h = rloc[r]
            entries.append(dict(r=r, c=c, p=par[c], slot=slot[c], pair=k,
                               half=h, start=(first[(r, par[c])] == i),
                               stop=(last[(r, par[c])] == i)))
        # accumulators with no blocks -> memset
        empty = []
        for (k, h), r in regions.items():
            for p in (0, 1):
                if not any(par[c] == p for c in rows[r]):
                    empty.append((k, h, p))
        chunks.append(dict(rows=rs, regions=regions, entries=entries,
                           empty=empty))

    return dict(cells=cells, chunks=chunks, pair=pair, nslot=nslot)


def _build_images(plan, blocks, bias):
    cells = plan["cells"]
    summed = {}
    for key, ks in cells.items():
        if len(ks) == 1:
            summed[key] = blocks[ks[0]]
        else:
            acc = blocks[ks[0]].astype(np.float32).copy()
            for k in ks[1:]:
                acc += blocks[k]
            summed[key] = acc

    # one interleaved image: per chunk, parity-0 blocks on partitions 0-63
    # and parity-1 on 64-127, column-aligned and zero padded to the wider half
    widths = []
    for ch in plan["chunks"]:
        n_e = sum(1 for e in ch["entries"] if e["p"] == 0)
        n_o = len(ch["entries"]) - n_e
        widths.append(max(n_e, n_o, 1))
    wtot = sum(widths)
    img = np.zeros((128, wtot * BS), np.float32)
    seg = []  # per chunk: (col offset in blocks, width in blocks)
    off = 0
    for wch, ch in zip(widths, plan["chunks"]):
        ie = io = 0
        for e in ch["entries"]:
            B = summed[(e["r"], e["c"])]
            if e["p"] == 0:
                img[0:64, (off + ie) * BS:(off + ie + 1) * BS] = B.T
                e["loc"] = ie
                ie += 1
            else:
                img[64:128, (off + io) * BS:(off + io + 1) * BS] = B.T
                e["loc"] = io
                io += 1
        seg.append((off, wch))
        off += wch

    bias_img = np.zeros((128, NCHUNK), np.float32)
    for ci, ch in enumerate(plan["chunks"]):
        for (k, h), r in ch["regions"].items():
            bias_img[h * 64:(h + 1) * 64, ci] = bias[r * BS:(r + 1) * BS]

    return img.astype(ml_dtypes.bfloat16), bias_img, seg


def _split_excess_waits(nc, mybir, limit=1):
    n = 0
    for fn in nc.m.functions:
        for bb in fn.blocks:
            out = []
            for inst in bb.instructions:
                si = inst.sync_info
                if si is not None and si.on_wait and len(si.on_wait) > limit:
                    waits = list(si.on_wait)
                    ups = list(si.on_update)
                    for j, w in enumerate(waits[:-limit]):
                        nop = mybir.InstNoOp(name=f"{inst.name}-ws{j}", ins=[], outs=[])
                        nop.engine = inst.engine
                        nop.sync_info = mybir.SyncInfo(on_wait=[w], on_update=[])
                        out.append(nop)
                        n += 1
                    inst.sync_info = mybir.SyncInfo(on_wait=waits[-limit:], on_update=ups)
                out.append(inst)
            bb.instructions = out
    return n


def _thin_engine_sem_updates(nc, mybir, engines=("EngineType.PE",)):
    """Drop per-instruction +1 sem increments that no wait ever observes.

    Tile gives every engine instruction a `then_inc(engine_sem)`; on the PE a
    serialized EVT_SEM write costs ~26 ns per matmul. Only ticks some other
    instruction actually waits on are needed, so keep increments just before
    each waited tick and renumber all waits by rank.
    """
    insts = []
    for fn in nc.m.functions:
        for bb in fn.blocks:
            insts.extend(bb.instructions)

    from collections import defaultdict
    upd_insts = defaultdict(list)   # sem id -> [instruction, ...] program order
    upd_ok = defaultdict(lambda: True)
    upd_engine = {}
    waited = defaultdict(set)       # sem id -> waited values
    wait_ok = defaultdict(lambda: True)
    for inst in insts:
        si = inst.sync_info
        if si is None:
            continue
        for u in si.on_update:
            if u.sync_type != "semaphore":
                continue
            if u.update_mode != "sem-inc" or u.update_value != 1:
                upd_ok[u.id] = False
            e = str(inst.engine)
            if u.id in upd_engine and upd_engine[u.id] != e:
                upd_ok[u.id] = False
            upd_engine[u.id] = e
            upd_insts[u.id].append(inst)
        for w in si.on_wait:
            if w.sync_type != "semaphore":
                continue
            if w.wait_mode != "sem-ge-imm" or w.wait_reg is not None:
                wait_ok[w.id] = False
            waited[w.id].add(w.wait_value)

    victims = [s for s, il in upd_insts.items()
               if upd_ok[s] and wait_ok[s] and upd_engine.get(s) in engines
               and len(il) > 8]
    for s in victims:
        il = upd_insts[s]
        W = sorted(v for v in waited.get(s, set()) if 1 <= v <= len(il))
        keep_ticks = set(W)
        rank = {v: i + 1 for i, v in enumerate(W)}
        # always keep the final tick so the kernel tail drain can await it
        if len(il) not in keep_ticks:
            keep_ticks.add(len(il))
            rank[len(il)] = len(W) + 1
        for t, inst in enumerate(il, start=1):
            si = inst.sync_info
            ups = [u for u in si.on_update
                   if not (u.sync_type == "semaphore" and u.id == s)]
            if t in keep_ticks:
                ups.append(mybir.SyncUpdate(
                    sync_type="semaphore", id=s, ant_name=f"thin{s}",
                    update_mode="sem-inc", update_value=1, update_reg=None))
            inst.sync_info = mybir.SyncInfo(on_wait=list(si.on_wait),
                                            on_update=ups)
        # renumber waits on this sem everywhere
        for inst in insts:
            si = inst.sync_info
            if si is None or not si.on_wait:
                continue
            changed = False
            ws = []
            for w in si.on_wait:
                if w.sync_type == "semaphore" and w.id == s:
                    nv = rank.get(w.wait_value)
                    if nv is None:
                        nv = sum(1 for v in rank if v <= w.wait_value)
                    ws.append(mybir.SyncWait(
                        sync_type="semaphore", id=s, ant_name=f"thin{s}",
                        wait_mode="sem-ge-imm", wait_value=nv, wait_reg=None))
                    changed = True
                else:
                    ws.append(w)
            if changed:
                inst.sync_info = mybir.SyncInfo(on_wait=ws,
                                                on_update=list(si.on_update))
    return victims


def _build_bass(plan, wimg, seg, nslot):
    import concourse.bass as bass
    import concourse.mybir as mybir
    import concourse.tile as tile

    F32 = mybir.dt.float32
    BF16 = mybir.dt.bfloat16

    nc = bass.Bass()
    xTd = nc.declare_dram_parameter("xT", [128, nslot * NL], BF16, isOutput=False)
    imd = nc.declare_dram_parameter("img", [128, wimg * BS], BF16, isOutput=False)
    bd = nc.declare_dram_parameter("bias_img", [128, NCHUNK], F32, isOutput=False)
    yTd = nc.declare_dram_parameter("yT", [128, NCHUNK * NL], BF16, isOutput=True)

    nxb = (nslot + XB - 1) // XB
    # weight batches: pairs for the first chunks, then fours
    wbounds = [0, 2, 4] + list(range(8, NCHUNK, 4)) + [NCHUNK]
    # output batches: fours, with a small tail for a short drain->store chain
    obounds = list(range(0, NCHUNK - 4, OB)) + [NCHUNK - 4, NCHUNK - 2,
                                               NCHUNK - 1, NCHUNK]
    ob_of = {}
    for b in range(len(obounds) - 1):
        for ci in range(obounds[b], obounds[b + 1]):
            ob_of[ci] = b

    def wslice(b):
        c0, c1 = wbounds[b], wbounds[b + 1] - 1
        return seg[c0][0] * BS, (seg[c1][0] + seg[c1][1]) * BS

    def xslice(b):
        return b * XB * NL, min(nslot, (b + 1) * XB) * NL

    with tile.TileContext(nc) as tc:
        with (
            tc.tile_pool(name="big", bufs=1) as big_pool,
            tc.tile_pool(name="cst", bufs=1) as cst_pool,
            tc.tile_pool(name="stp", bufs=3) as st_pool,
            tc.tile_pool(name="tmp", bufs=4) as tmp_pool,
            tc.tile_pool(name="ps", bufs=1, space="PSUM") as ps_pool,
        ):
            zblk = cst_pool.tile([128, BS], BF16, tag="zblk")
            nc.vector.memset(zblk[:], 0.0)
            wscr = cst_pool.tile([128, NL], BF16, tag="wscr")
            nc.vector.memset(wscr[:], 0.0)
            bias_t = cst_pool.tile([128, NCHUNK], F32, tag="bias")

            xt_t = big_pool.tile([128, nslot * NL], BF16, tag="xt")
            wt_t = big_pool.tile([128, wimg * BS], BF16, tag="wt")

            # inbound loads across both HWDGE queues (each drains FIFO,
            # both share HBM bandwidth at packet granularity): sync takes
            # the first x batch + the weight stream, scalar the rest of x
            # (it drains before output stores need that queue)
            sync_q = [("x", 0)] + [("w", b) for b in range(len(wbounds) - 1)]
            scal_q = ([("x", 1), ("x", 2), ("x", 3), ("x", 4), ("bias", 0)]
                      + [("x", b) for b in range(5, nxb)])
            for q, eng in ((sync_q, nc.sync), (scal_q, nc.scalar)):
                for kind, b in q:
                    if kind == "x":
                        if b >= nxb:
                            continue
                        lo, hi = xslice(b)
                        eng.dma_start(out=xt_t[:, lo:hi], in_=xTd[:, lo:hi])
                    elif kind == "w":
                        lo, hi = wslice(b)
                        eng.dma_start(out=wt_t[:, lo:hi], in_=imd[:, lo:hi])
                    else:
                        eng.dma_start(out=bias_t[:], in_=bd[:])

            # warmup matmuls on zero tiles: keep the PE busy through the
            # load phase so the p-state/HAM ramp is done when real work lands
            ps_warm = ps_pool.tile([128, NL], F32, tag="bank6", name="warm")
            for i in range(NWARM):
                h = i % 2
                nc.tensor.matmul(ps_warm[h * 64:(h + 1) * 64, :],
                                 zblk[0:64, :], wscr[0:64, :],
                                 start=True, stop=True,
                                 tile_position=(0, h * 64))

            for ci, ch in enumerate(plan["chunks"]):
                coff = seg[ci][0]
                boff = 2 * (ci % 4)
                ps_tiles = [ps_pool.tile([128, NL], F32, tag=f"bank{boff+b}",
                                         name=f"ps{ci}_{b}")
                            for b in range(2)]
                for (k, h, p) in ch["empty"]:
                    nc.tensor.matmul(
                        ps_tiles[p][h * 64:(h + 1) * 64, :],
                        zblk[p * 64:(p + 1) * 64, :],
                        xt_t[p * 64:(p + 1) * 64, 0:NL],
                        start=True, stop=True,
                        tile_position=(p * 64, h * 64))

                for e in ch["entries"]:
                    p = e["p"]
                    lhsT = wt_t[p * 64:(p + 1) * 64,
                                (coff + e["loc"]) * BS:(coff + e["loc"] + 1) * BS]
                    rhs = xt_t[p * 64:(p + 1) * 64,
                               e["slot"] * NL:(e["slot"] + 1) * NL]
                    out = ps_tiles[p][e["half"] * 64:(e["half"] + 1) * 64, :]
                    nc.tensor.matmul(out, lhsT, rhs, start=e["start"],
                                     stop=e["stop"],
                                     tile_position=(p * 64, e["half"] * 64))

                b = ob_of[ci]
                if ci == obounds[b]:
                    st_t = st_pool.tile([128, OB * NL], BF16, tag="st",
                                        name=f"st{b}")
                tmp = tmp_pool.tile([128, NL], F32, tag="tmp",
                                    name=f"tmp{ci}")
                soff = (ci - obounds[b]) * NL
                # drain in half-columns: shorter ACT->DVE latency chain
                for lo, hi in ((0, NL // 2), (NL // 2, NL)):
                    nc.scalar.activation(
                        tmp[:, lo:hi], ps_tiles[0][:, lo:hi],
                        mybir.ActivationFunctionType.Identity,
                        bias=bias_t[:, ci:ci + 1])
                    nc.vector.tensor_tensor(st_t[:, soff + lo:soff + hi],
                                            tmp[:, lo:hi],
                                            ps_tiles[1][:, lo:hi],
                                            op=mybir.AluOpType.add)
                if ci == obounds[b + 1] - 1:
                    n = obounds[b + 1] - obounds[b]
                    nc.scalar.dma_start(
                        out=yTd[:, obounds[b] * NL:obounds[b + 1] * NL],
                        in_=st_t[:, :n * NL])

    _thin_engine_sem_updates(nc, mybir)
    _split_excess_waits(nc, mybir)
    return nc


def kernel(x, blocks, bias, row_idx, col_idx):
    from concourse.bass_utils import run_bass_kernel_spmd

    row_idx = np.asarray(row_idx)
    col_idx = np.asarray(col_idx)
    key = (row_idx.tobytes(), col_idx.tobytes())
    if key not in _CACHE:
        _CACHE[key] = [_plan(row_idx, col_idx), None]
    plan = _CACHE[key][0]

    img, bias_img, seg = _build_images(plan, np.asarray(blocks),
                                       np.asarray(bias, np.float32))
    if _CACHE[key][1] is None:
        _CACHE[key][1] = _build_bass(plan, img.shape[1] // BS, seg,
                                     plan["nslot"])
    nc = _CACHE[key][1]

    # feature row order of the xT image: slot s = (pair[s][0] block on
    # partitions 0-63, pair[s][1] on 64-127)
    feat = np.empty((plan["nslot"], 128), np.int64)
    for s, (a, b) in enumerate(plan["pair"]):
        feat[s, :64] = np.arange(a * BS, (a + 1) * BS)
        feat[s, 64:] = np.arange(b * BS, (b + 1) * BS)

    x = np.asarray(x)
    in_maps = []
    for i in range(NCORES):
        xT = x[i * NL:(i + 1) * NL, :].T.astype(ml_dtypes.bfloat16)
        ximg = np.ascontiguousarray(
            xT[feat.reshape(-1)].reshape(plan["nslot"], 128, NL)
            .swapaxes(0, 1).reshape(128, plan["nslot"] * NL))
        in_maps.append({"xT": ximg, "img": img, "bias_img": bias_img})

    res = run_bass_kernel_spmd(nc, in_maps, list(range(NCORES))).results

    y = np.empty((N_TOK, OUT_F), np.float32)
    for i in range(NCORES):
        raw = np.asarray(res[i]["yT"]).astype(np.float32)
        yl = y[i * NL:(i + 1) * NL]
        for ci, ch in enumerate(plan["chunks"]):
            for (k, h), r in ch["regions"].items():
                yl[:, r * BS:(r + 1) * BS] = \
                    raw[h * 64:(h + 1) * 64, ci * NL:(ci + 1) * NL].T
    return y
